# revision 1
# baseline (speedup 1.0000x reference)
"""Trainium2 Bass kernel for MultiKeyframeProcessor (linear keyframe interpolation).

Problem: latents [K=5,B=1,C=128,H=32,W=32], strengths [K], frame_indices [K]
(sorted, unique, < T=257). Output: (conditioning_latents [1,128,257,32,32],
conditioning_masks [1,257]).

Every output frame t is  out[t] = lat[j] + alpha_t * (lat[j+1] - lat[j])  for
its bracketing keyframe pair j(t) (alpha=0 for copies: before-first /
at-keyframe / after-last frames). SPMD over 8 cores runs ONE program; frames
are assigned to (core, local-slot) grouped by bracketing pair so the per-slot
pair index is compile-time static and identical across cores, while alpha is
per-core input data. The host reassembles per-core outputs into the full T
axis (pure index permutation, no arithmetic).
"""

import math
from contextlib import ExitStack

import numpy as np

T = 257
K = 5
C = 128
HW = 1024  # 32*32
NCORES = 8
G = 8  # frames per output DMA chunk


def _plan_frames(fi):
    """Classify every frame and assign frames to per-core slots.

    Returns (plan, per_core_t, alpha, coef):
      plan: list over local slots f of ('copy', j) or ('interp', j) — static,
            identical for all cores.
      per_core_t: [NCORES][N] global frame index handled by (core, slot).
      alpha: [T] float32 interpolation weight per global frame.
      coef: [K, T] float32 mask coefficients: mask[t] = sum_k coef[k,t]*strengths[k].
    """
    fi = np.asarray(fi, dtype=np.int64)
    alpha = np.zeros(T, np.float32)
    coef = np.zeros((K, T), np.float32)
    groups = {}  # (kind, j) -> list of t

    for t in range(T):
        tf = np.float32(t)
        if t < fi[0]:
            g = ("copy", 0)
            fi0 = np.float32(fi[0])
            decay = max(np.float32(1.0) - (fi0 - tf) / max(fi0, np.float32(1.0)),
                        np.float32(0.0))
            coef[0, t] = decay
        elif t > fi[-1]:
            g = ("copy", K - 1)
            fi4 = np.float32(fi[-1])
            decay = max(np.float32(1.0) - (tf - fi4) / (np.float32(T) - fi4),
                        np.float32(0.0))
            coef[K - 1, t] = decay
        else:
            j = int(np.searchsorted(fi, t, side="right")) - 1
            if fi[j] == t:
                g = ("copy", j)
                coef[j, t] = np.float32(1.0)
            else:
                g = ("interp", j)
                a = np.float32(np.float32(t - fi[j]) /
                               np.float32(max(fi[j + 1] - fi[j], 1)))
                alpha[t] = a
                coef[j, t] = np.float32(1.0) - a
                coef[j + 1, t] = a
        groups.setdefault(g, []).append(t)

    order = []
    for j in range(K):
        if ("copy", j) in groups:
            order.append(("copy", j))
        if j < K - 1 and ("interp", j) in groups:
            order.append(("interp", j))

    plan = []
    per_core_t = [[] for _ in range(NCORES)]
    for g in order:
        lst = groups[g]
        c = len(lst)
        n = math.ceil(c / NCORES)
        plan.extend([g] * n)
        for m in range(NCORES):
            for idx in range(n):
                gi = m * n + idx
                per_core_t[m].append(lst[min(gi, c - 1)])
    return plan, per_core_t, alpha, coef


def _build_program(plan, N):
    """Build the (single, SPMD) Bass program for N local frames."""
    from concourse import bacc, mybir
    import concourse.tile as tile

    f32 = mybir.dt.float32
    Alu = mybir.AluOpType

    nc = bacc.Bacc("TRN2", target_bir_lowering=False, debug=False,
                   num_devices=NCORES)
    lat_d = nc.dram_tensor("lat", [C, K * HW], f32, kind="ExternalInput")
    alpha_d = nc.dram_tensor("alpha", [C, N], f32, kind="ExternalInput")
    coef_d = nc.dram_tensor("coef", [1, K * N], f32, kind="ExternalInput")
    str_d = nc.dram_tensor("strengths", [1, K], f32, kind="ExternalInput")
    out_d = nc.dram_tensor("out", [C, N * HW], f32, kind="ExternalOutput")
    mask_d = nc.dram_tensor("mask", [1, N], f32, kind="ExternalOutput")

    interp_js = sorted({j for kind, j in plan if kind == "interp"})

    with tile.TileContext(nc) as tc, ExitStack() as ctx:
        const = ctx.enter_context(tc.tile_pool(name="const", bufs=1))
        outp = ctx.enter_context(tc.tile_pool(name="outp", bufs=3))

        lat_sb = const.tile([C, K * HW], f32)
        nc.sync.dma_start(out=lat_sb[:], in_=lat_d.ap())
        alpha_sb = const.tile([C, N], f32)
        nc.sync.dma_start(out=alpha_sb[:], in_=alpha_d.ap())
        coef_sb = const.tile([1, K * N], f32)
        nc.sync.dma_start(out=coef_sb[:], in_=coef_d.ap())
        str_sb = const.tile([1, K], f32)
        nc.sync.dma_start(out=str_sb[:], in_=str_d.ap())

        # Per-pair deltas d_j = lat[j+1] - lat[j]
        d_sb = const.tile([C, (K - 1) * HW], f32)
        for j in interp_js:
            nc.vector.tensor_tensor(
                out=d_sb[:, j * HW:(j + 1) * HW],
                in0=lat_sb[:, (j + 1) * HW:(j + 2) * HW],
                in1=lat_sb[:, j * HW:(j + 1) * HW],
                op=Alu.subtract,
            )

        # Frame computation, chunked for output DMA
        for c0 in range(0, N, G):
            g_sz = min(G, N - c0)
            otile = outp.tile([C, G * HW], f32, tag="ot")
            for g in range(g_sz):
                f = c0 + g
                kind, j = plan[f]
                dst = otile[:, g * HW:(g + 1) * HW]
                if kind == "copy":
                    nc.scalar.copy(out=dst, in_=lat_sb[:, j * HW:(j + 1) * HW])
                else:
                    nc.vector.scalar_tensor_tensor(
                        out=dst,
                        in0=d_sb[:, j * HW:(j + 1) * HW],
                        scalar=alpha_sb[:, f:f + 1],
                        in1=lat_sb[:, j * HW:(j + 1) * HW],
                        op0=Alu.mult,
                        op1=Alu.add,
                    )
            nc.sync.dma_start(out=out_d.ap()[:, c0 * HW:(c0 + g_sz) * HW],
                              in_=otile[:, :g_sz * HW])

        # Masks: mask[f] = sum_k coef[k,f] * strengths[k]
        prod = const.tile([1, K * N], f32)
        for k in range(K):
            nc.vector.tensor_scalar(
                out=prod[0:1, k * N:(k + 1) * N],
                in0=coef_sb[0:1, k * N:(k + 1) * N],
                scalar1=str_sb[0:1, k:k + 1],
                scalar2=None,
                op0=Alu.mult,
            )
        m01 = const.tile([1, N], f32)
        nc.vector.tensor_add(m01[:], prod[0:1, 0:N], prod[0:1, N:2 * N])
        m23 = const.tile([1, N], f32)
        nc.vector.tensor_add(m23[:], prod[0:1, 2 * N:3 * N], prod[0:1, 3 * N:4 * N])
        m03 = const.tile([1, N], f32)
        nc.vector.tensor_add(m03[:], m01[:], m23[:])
        mask_sb = const.tile([1, N], f32)
        nc.vector.tensor_add(mask_sb[:], m03[:], prod[0:1, 4 * N:5 * N])
        nc.sync.dma_start(out=mask_d.ap(), in_=mask_sb[:])

    nc.compile()
    return nc


LAST_PERF = None  # BassKernelResults of the most recent run (set when _trace)


def kernel(latents, strengths, frame_indices, _trace=False):
    from concourse.bass_utils import run_bass_kernel_spmd

    global LAST_PERF
    latents = np.asarray(latents, dtype=np.float32)
    strengths = np.asarray(strengths, dtype=np.float32)
    frame_indices = np.asarray(frame_indices)

    plan, per_core_t, alpha, coef = _plan_frames(frame_indices)
    N = len(plan)

    # [C, K*HW]: partition row c holds all 5 keyframes' (contiguous) hw planes
    lat_h = np.ascontiguousarray(
        latents[:, 0].reshape(K, C, HW).transpose(1, 0, 2).reshape(C, K * HW))
    str_h = np.ascontiguousarray(strengths.reshape(1, K))

    in_maps = []
    for m in range(NCORES):
        idx = np.asarray(per_core_t[m], dtype=np.int64)
        alpha_m = np.ascontiguousarray(
            np.broadcast_to(alpha[idx][None, :], (C, N)))
        coef_m = np.ascontiguousarray(coef[:, idx].reshape(1, K * N))
        in_maps.append({
            "lat": lat_h,
            "alpha": alpha_m,
            "coef": coef_m,
            "strengths": str_h,
        })

    nc = _build_program(plan, N)
    res = run_bass_kernel_spmd(nc, in_maps, core_ids=list(range(NCORES)),
                               trace=_trace)
    if _trace:
        LAST_PERF = res

    full = np.empty((C, T, HW), np.float32)
    mask_full = np.empty(T, np.float32)
    for m in range(NCORES):
        idx = np.asarray(per_core_t[m], dtype=np.int64)
        full[:, idx, :] = res.results[m]["out"].reshape(C, N, HW)
        mask_full[idx] = res.results[m]["mask"].reshape(N)

    conditioning_latents = full.reshape(1, C, T, 32, 32)
    conditioning_masks = np.ascontiguousarray(
        np.broadcast_to(mask_full[None, :], (1, T)))
    return conditioning_latents, conditioning_masks


# revision 22
# speedup vs baseline: 1268.0403x; 1268.0403x over previous
"""Trainium2 Bass kernel for MultiKeyframeProcessor (linear keyframe interpolation).

Problem: latents [K=5,B=1,C=128,H=32,W=32], strengths [K], frame_indices [K]
(sorted, unique, < T=257). Output: (conditioning_latents [1,128,257,32,32],
conditioning_masks [1,257]).

Every output frame t is  out[t] = lat[j] + alpha_t * (lat[j+1] - lat[j])  for
its bracketing keyframe pair j(t) in {0..3}; frames at/before the first
keyframe are exact copies of lat[0] ("Z"), frames at/after the last keyframe
exact copies of lat[4] ("C") — those are written by pure DMA from SBUF with
no compute, which also fills the DMA pipe while the vector engine ramps.

SPMD over 8 cores runs ONE program; frames are assigned to (core, local-slot)
grouped by kind so the per-slot op is compile-time static and identical
across cores, while alpha is per-core input data. The host reassembles the
per-core outputs into the full T axis (index permutation only).
"""

import math
from contextlib import ExitStack

import numpy as np

T = 257
K = 5
C = 128
HW = 1024  # 32*32
NCORES = 8
NPAIR = K - 1


def _plan_frames(fi):
    """Classify frames and assign them to per-core slots.

    Returns (plan, per_core_t, alpha, coef):
      plan: list over local slots of "Z" (copy lat0), "C" (copy lat4) or pair
            index j — static, same for all cores.
      per_core_t: [NCORES][N] global frame index handled by (core, slot).
      alpha: [T] float32 interpolation weight per global frame.
      coef: [K, T] float32 mask coefficients: mask[t] = sum_k coef[k,t]*strengths[k].
    """
    fi = np.asarray(fi, dtype=np.int64)
    alpha = np.zeros(T, np.float32)
    coef = np.zeros((K, T), np.float32)

    # --- mask coefficients (independent of latent grouping) ---
    for t in range(T):
        tf = np.float32(t)
        if t < fi[0]:
            fi0 = np.float32(fi[0])
            coef[0, t] = max(
                np.float32(1.0) - (fi0 - tf) / max(fi0, np.float32(1.0)),
                np.float32(0.0))
        elif t > fi[-1]:
            fi4 = np.float32(fi[-1])
            coef[K - 1, t] = max(
                np.float32(1.0) - (tf - fi4) / (np.float32(T) - fi4),
                np.float32(0.0))
        else:
            j = int(np.searchsorted(fi, t, side="right")) - 1
            if fi[j] == t:
                coef[j, t] = np.float32(1.0)
            else:
                a = np.float32(np.float32(t - fi[j]) /
                               np.float32(max(fi[j + 1] - fi[j], 1)))
                coef[j, t] = np.float32(1.0) - a
                coef[j + 1, t] = a

    # --- latent grouping with flexible boundary frames ---
    # p1 = frames t<=fi0 (lat0 copies): group Z (pure DMA) or int0 (alpha=0).
    # p2 = frames t>=fi4 (lat4 copies): group C (pure DMA) or int3 (alpha=1).
    # t==fi_j (0<j<4): int_{j-1} (alpha=1) or int_j (alpha=0).
    # Interior frames are fixed to their pair. Search the (mod-8) boundary
    # choices for the minimum total slot count.
    p1 = [t for t in range(T) if t <= fi[0]]
    p2 = [t for t in range(T) if t >= fi[-1]]
    interior = [[] for _ in range(NPAIR)]
    for t in range(fi[0] + 1, fi[-1]):
        j = int(np.searchsorted(fi, t, side="right")) - 1
        if fi[j] != t:
            interior[j].append(t)

    def slots(n):
        return math.ceil(n / NCORES)

    best = None
    for zc in range(len(p1), max(-1, len(p1) - NCORES), -1):
        for cc in range(len(p2), max(-1, len(p2) - NCORES), -1):
            for bits in range(8):  # bit j-1 set -> fi_j goes to int_{j-1}
                n_int = [len(interior[j]) for j in range(NPAIR)]
                n_int[0] += len(p1) - zc
                n_int[NPAIR - 1] += len(p2) - cc
                for j in (1, 2, 3):
                    if bits >> (j - 1) & 1:
                        n_int[j - 1] += 1
                    else:
                        n_int[j] += 1
                total = slots(zc) + slots(cc) + sum(slots(n) for n in n_int)
                key = (total, -(zc + cc), -zc)
                if best is None or key < best[0]:
                    best = (key, zc, cc, bits)
    _, zc, cc, bits = best

    zgrp = p1[:zc]
    igrp = [list(g) for g in interior]
    igrp[0] = p1[zc:] + igrp[0]          # alpha = 0
    cgrp = p2[len(p2) - cc:]
    for t in p2[:len(p2) - cc]:          # alpha = 1
        alpha[t] = np.float32(1.0)
        igrp[NPAIR - 1].append(t)
    for j in (1, 2, 3):
        if bits >> (j - 1) & 1:
            alpha[fi[j]] = np.float32(1.0)
            igrp[j - 1].append(int(fi[j]))
        else:
            igrp[j].insert(0, int(fi[j]))  # alpha = 0
    for j in range(NPAIR):
        for t in igrp[j]:
            if fi[j] < t < fi[j + 1]:
                alpha[t] = np.float32(np.float32(t - fi[j]) /
                                      np.float32(max(fi[j + 1] - fi[j], 1)))
        igrp[j].sort()

    plan = []
    per_core_t = [[] for _ in range(NCORES)]
    for key, lst in [("Z", zgrp), ("C", cgrp)] + list(enumerate(igrp)):
        c = len(lst)
        if c == 0:
            continue
        n = math.ceil(c / NCORES)
        plan.extend([key] * n)
        for m in range(NCORES):
            for idx in range(n):
                gi = m * n + idx
                per_core_t[m].append(lst[min(gi, c - 1)])
    return plan, per_core_t, alpha, coef


def _chunk_sizes(n):
    """stt-slot chunking: moderate leading chunk (copy-slot DMAs cover the
    ramp), 8-frame middle chunks, small tail chunks so the last DMA is short."""
    sizes = []
    if n > 0:
        sizes.append(min(4, n))
        n -= sizes[-1]
    while n > 7:
        sizes.append(8)
        n -= 8
    while n > 0:
        s = min(4, n)
        sizes.append(s)
        n -= s
    return sizes


def _build_program(plan, N, reps=1):
    """Build the (single, SPMD) Bass program for N local frames.

    reps>1 repeats the frame work (overwriting the same output) — used only
    by the dev timing harness to measure per-iteration HW time via slope.
    """
    from concourse import bacc, mybir
    import concourse.tile as tile

    f32 = mybir.dt.float32
    Alu = mybir.AluOpType

    nc = bacc.Bacc("TRN2", target_bir_lowering=False, debug=False,
                   num_devices=NCORES)
    lat_d = nc.dram_tensor("lat", [C, K * HW], f32, kind="ExternalInput")
    alpha_d = nc.dram_tensor("alpha", [C, N], f32, kind="ExternalInput")
    coef_d = nc.dram_tensor("coef", [1, K * N], f32, kind="ExternalInput")
    str_d = nc.dram_tensor("strengths", [1, K], f32, kind="ExternalInput")
    out_d = nc.dram_tensor("out", [C, N * HW], f32, kind="ExternalOutput")
    mask_d = nc.dram_tensor("mask", [1, N], f32, kind="ExternalOutput")

    pair_js = sorted({j for j in plan if isinstance(j, int)})

    with tile.TileContext(nc) as tc, ExitStack() as ctx:
        const = ctx.enter_context(tc.tile_pool(name="const", bufs=1))
        outp = ctx.enter_context(tc.tile_pool(name="outp", bufs=3))

        # alpha is read by every stt op — tiny, load first
        alpha_sb = const.tile([C, N], f32)
        nc.sync.dma_start(out=alpha_sb[:], in_=alpha_d.ap())

        # Keyframe latents: lat0+lat1 first (unblocks "Z" copies and d0),
        # then the rest in one transfer.
        lat01 = const.tile([C, 2 * HW], f32)
        nc.sync.dma_start(out=lat01[:], in_=lat_d.ap()[:, 0:2 * HW])
        lat_rest = const.tile([C, (K - 2) * HW], f32)
        nc.sync.dma_start(out=lat_rest[:], in_=lat_d.ap()[:, 2 * HW:])

        def lat_ap(k):
            if k < 2:
                return lat01[:, k * HW:(k + 1) * HW]
            return lat_rest[:, (k - 2) * HW:(k - 1) * HW]

        dma_engines = [nc.scalar, nc.sync]
        n_dma = 0

        # Masks first (tiny; DVE does them while the lat loads are in
        # flight): mask[f] = sum_k coef[k,f] * strengths[k]
        coef_sb = const.tile([1, K * N], f32)
        nc.gpsimd.dma_start(out=coef_sb[:], in_=coef_d.ap())
        str_sb = const.tile([1, K], f32)
        nc.gpsimd.dma_start(out=str_sb[:], in_=str_d.ap())
        prod = const.tile([1, K * N], f32)
        for k in range(K):
            nc.vector.tensor_scalar(
                out=prod[0:1, k * N:(k + 1) * N],
                in0=coef_sb[0:1, k * N:(k + 1) * N],
                scalar1=str_sb[0:1, k:k + 1],
                scalar2=None,
                op0=Alu.mult,
            )
        m01 = const.tile([1, N], f32)
        nc.vector.tensor_add(m01[:], prod[0:1, 0:N], prod[0:1, N:2 * N])
        m23 = const.tile([1, N], f32)
        nc.vector.tensor_add(m23[:], prod[0:1, 2 * N:3 * N], prod[0:1, 3 * N:4 * N])
        m03 = const.tile([1, N], f32)
        nc.vector.tensor_add(m03[:], m01[:], m23[:])
        mask_sb = const.tile([1, N], f32)
        nc.vector.tensor_add(mask_sb[:], m03[:], prod[0:1, 4 * N:5 * N])
        nc.gpsimd.dma_start(out=mask_d.ap(), in_=mask_sb[:])

        # stt slots, chunked for output DMA. Deltas d_j = lat[j+1] - lat[j]
        # are computed just-in-time: early pairs on DVE (in issue order,
        # right before their first stt), later pairs on Pool (idle early).
        d_t = {}
        d_pool = ctx.enter_context(tc.tile_pool(name="deltas", bufs=1))
        tmp_pool = ctx.enter_context(tc.tile_pool(name="offtmp", bufs=3))

        def get_d(j):
            if j not in d_t:
                dt_ = d_pool.tile([C, HW], f32, name=f"d{j}", tag=f"d{j}")
                eng = nc.vector if j < 2 else nc.gpsimd
                eng.tensor_tensor(out=dt_[:], in0=lat_ap(j + 1),
                                  in1=lat_ap(j), op=Alu.subtract)
                d_t[j] = dt_
            return d_t[j]

        # Measured per-[128,1024]-op rates: DVE stt ~0.75us, ACT mul ~0.75us,
        # Pool add ~2.07us. Offload a fraction of frames to the ACT->Pool
        # 2-op path so DVE and Pool finish together.
        stt_all = [f for f, key in enumerate(plan) if not isinstance(key, str)]
        n_off = int(round(len(stt_all) * 0.75 / (0.75 + 2.07)))
        off_slots = set(stt_all[::max(1, len(stt_all) // n_off)][:n_off]
                        if n_off else [])

        for _rep in range(reps):
            # Copy slots: pure DMA SBUF->DRAM from the lat tiles (no
            # compute). These fill the DMA pipe while DVE ramps.
            stt_slots = []
            for f, key in enumerate(plan):
                if key == "Z":
                    src = lat_ap(0)
                elif key == "C":
                    src = lat_ap(K - 1)
                else:
                    stt_slots.append(f)
                    continue
                dma_engines[n_dma % 2].dma_start(
                    out=out_d.ap()[:, f * HW:(f + 1) * HW], in_=src)
                n_dma += 1

            ci = 0
            for g_sz in _chunk_sizes(len(stt_slots)):
                f0 = stt_slots[ci]
                otile = outp.tile([C, g_sz * HW], f32, tag="ot")
                for g in range(g_sz):
                    f = stt_slots[ci + g]
                    assert f == f0 + g
                    j = plan[f]
                    dst = otile[:, g * HW:(g + 1) * HW]
                    if f in off_slots:
                        tmp = tmp_pool.tile([C, HW], f32, tag="tmp")
                        nc.scalar.mul(tmp[:], get_d(j)[:], alpha_sb[:, f:f + 1])
                        nc.gpsimd.tensor_tensor(out=dst, in0=tmp[:],
                                                in1=lat_ap(j), op=Alu.add)
                    else:
                        nc.vector.scalar_tensor_tensor(
                            out=dst,
                            in0=get_d(j)[:],
                            scalar=alpha_sb[:, f:f + 1],
                            in1=lat_ap(j),
                            op0=Alu.mult,
                            op1=Alu.add,
                        )
                dma_engines[n_dma % 2].dma_start(
                    out=out_d.ap()[:, f0 * HW:(f0 + g_sz) * HW], in_=otile[:])
                n_dma += 1
                ci += g_sz


    nc.compile()
    return nc


LAST_PERF = None  # BassKernelResults of the most recent run (set when _trace)


def kernel(latents, strengths, frame_indices, _trace=False):
    from concourse.bass_utils import run_bass_kernel_spmd

    global LAST_PERF
    latents = np.asarray(latents, dtype=np.float32)
    strengths = np.asarray(strengths, dtype=np.float32)
    frame_indices = np.asarray(frame_indices)

    plan, per_core_t, alpha, coef = _plan_frames(frame_indices)
    N = len(plan)

    # [C, K*HW]: partition row c holds all 5 keyframes' (contiguous) hw planes
    lat_h = np.ascontiguousarray(
        latents[:, 0].reshape(K, C, HW).transpose(1, 0, 2).reshape(C, K * HW))
    str_h = np.ascontiguousarray(strengths.reshape(1, K))

    in_maps = []
    for m in range(NCORES):
        idx = np.asarray(per_core_t[m], dtype=np.int64)
        alpha_m = np.ascontiguousarray(
            np.broadcast_to(alpha[idx][None, :], (C, N)))
        coef_m = np.ascontiguousarray(coef[:, idx].reshape(1, K * N))
        in_maps.append({
            "lat": lat_h,
            "alpha": alpha_m,
            "coef": coef_m,
            "strengths": str_h,
        })

    nc = _build_program(plan, N)
    res = run_bass_kernel_spmd(nc, in_maps, core_ids=list(range(NCORES)),
                               trace=_trace)
    if _trace:
        LAST_PERF = res

    full = np.empty((C, T, HW), np.float32)
    mask_full = np.empty(T, np.float32)
    for m in range(NCORES):
        idx = np.asarray(per_core_t[m], dtype=np.int64)
        full[:, idx, :] = res.results[m]["out"].reshape(C, N, HW)
        mask_full[idx] = res.results[m]["mask"].reshape(N)

    conditioning_latents = full.reshape(1, C, T, 32, 32)
    conditioning_masks = np.ascontiguousarray(
        np.broadcast_to(mask_full[None, :], (1, T)))
    return conditioning_latents, conditioning_masks


# revision 25
# speedup vs baseline: 1287.1154x; 1.0150x over previous
"""Trainium2 Bass kernel for MultiKeyframeProcessor (linear keyframe interpolation).

Problem: latents [K=5,B=1,C=128,H=32,W=32], strengths [K], frame_indices [K]
(sorted, unique, < T=257). Output: (conditioning_latents [1,128,257,32,32],
conditioning_masks [1,257]).

Every output frame t is  out[t] = lat[j] + alpha_t * (lat[j+1] - lat[j])  for
its bracketing keyframe pair j(t) in {0..3}; frames at/before the first
keyframe are exact copies of lat[0] ("Z"), frames at/after the last keyframe
exact copies of lat[4] ("C") — those are written by pure DMA from SBUF with
no compute, which also fills the DMA pipe while the vector engine ramps.

SPMD over 8 cores runs ONE program; frames are assigned to (core, local-slot)
grouped by kind so the per-slot op is compile-time static and identical
across cores, while alpha is per-core input data. The host reassembles the
per-core outputs into the full T axis (index permutation only).
"""

import math
from contextlib import ExitStack

import numpy as np

T = 257
K = 5
C = 128
HW = 1024  # 32*32
NCORES = 8
NPAIR = K - 1


def _plan_frames(fi):
    """Classify frames and assign them to per-core slots.

    Returns (plan, per_core_t, alpha, coef):
      plan: list over local slots of "Z" (copy lat0), "C" (copy lat4) or pair
            index j — static, same for all cores.
      per_core_t: [NCORES][N] global frame index handled by (core, slot).
      alpha: [T] float32 interpolation weight per global frame.
      coef: [K, T] float32 mask coefficients: mask[t] = sum_k coef[k,t]*strengths[k].
    """
    fi = np.asarray(fi, dtype=np.int64)
    alpha = np.zeros(T, np.float32)
    coef = np.zeros((K, T), np.float32)

    # --- mask coefficients (independent of latent grouping) ---
    for t in range(T):
        tf = np.float32(t)
        if t < fi[0]:
            fi0 = np.float32(fi[0])
            coef[0, t] = max(
                np.float32(1.0) - (fi0 - tf) / max(fi0, np.float32(1.0)),
                np.float32(0.0))
        elif t > fi[-1]:
            fi4 = np.float32(fi[-1])
            coef[K - 1, t] = max(
                np.float32(1.0) - (tf - fi4) / (np.float32(T) - fi4),
                np.float32(0.0))
        else:
            j = int(np.searchsorted(fi, t, side="right")) - 1
            if fi[j] == t:
                coef[j, t] = np.float32(1.0)
            else:
                a = np.float32(np.float32(t - fi[j]) /
                               np.float32(max(fi[j + 1] - fi[j], 1)))
                coef[j, t] = np.float32(1.0) - a
                coef[j + 1, t] = a

    # --- latent grouping with flexible boundary frames ---
    # p1 = frames t<=fi0 (lat0 copies): group Z (pure DMA) or int0 (alpha=0).
    # p2 = frames t>=fi4 (lat4 copies): group C (pure DMA) or int3 (alpha=1).
    # t==fi_j (0<j<4): int_{j-1} (alpha=1) or int_j (alpha=0).
    # Interior frames are fixed to their pair. Search the (mod-8) boundary
    # choices for the minimum total slot count.
    p1 = [t for t in range(T) if t <= fi[0]]
    p2 = [t for t in range(T) if t >= fi[-1]]
    interior = [[] for _ in range(NPAIR)]
    for t in range(fi[0] + 1, fi[-1]):
        j = int(np.searchsorted(fi, t, side="right")) - 1
        if fi[j] != t:
            interior[j].append(t)

    def slots(n):
        return math.ceil(n / NCORES)

    best = None
    for zc in range(len(p1), max(-1, len(p1) - NCORES), -1):
        for cc in range(len(p2), max(-1, len(p2) - NCORES), -1):
            for bits in range(8):  # bit j-1 set -> fi_j goes to int_{j-1}
                n_int = [len(interior[j]) for j in range(NPAIR)]
                n_int[0] += len(p1) - zc
                n_int[NPAIR - 1] += len(p2) - cc
                for j in (1, 2, 3):
                    if bits >> (j - 1) & 1:
                        n_int[j - 1] += 1
                    else:
                        n_int[j] += 1
                total = slots(zc) + slots(cc) + sum(slots(n) for n in n_int)
                key = (total, -(zc + cc), -zc)
                if best is None or key < best[0]:
                    best = (key, zc, cc, bits)
    _, zc, cc, bits = best

    zgrp = p1[:zc]
    igrp = [list(g) for g in interior]
    igrp[0] = p1[zc:] + igrp[0]          # alpha = 0
    cgrp = p2[len(p2) - cc:]
    for t in p2[:len(p2) - cc]:          # alpha = 1
        alpha[t] = np.float32(1.0)
        igrp[NPAIR - 1].append(t)
    for j in (1, 2, 3):
        if bits >> (j - 1) & 1:
            alpha[fi[j]] = np.float32(1.0)
            igrp[j - 1].append(int(fi[j]))
        else:
            igrp[j].insert(0, int(fi[j]))  # alpha = 0
    for j in range(NPAIR):
        for t in igrp[j]:
            if fi[j] < t < fi[j + 1]:
                alpha[t] = np.float32(np.float32(t - fi[j]) /
                                      np.float32(max(fi[j + 1] - fi[j], 1)))
        igrp[j].sort()

    plan = []
    per_core_t = [[] for _ in range(NCORES)]
    for key, lst in [("Z", zgrp), ("C", cgrp)] + list(enumerate(igrp)):
        c = len(lst)
        if c == 0:
            continue
        n = math.ceil(c / NCORES)
        plan.extend([key] * n)
        for m in range(NCORES):
            for idx in range(n):
                gi = m * n + idx
                per_core_t[m].append(lst[min(gi, c - 1)])
    return plan, per_core_t, alpha, coef


def _chunk_sizes(n):
    """stt-slot chunking: moderate leading chunk (copy-slot DMAs cover the
    ramp), 8-frame middle chunks, small tail chunks so the last DMA is short."""
    sizes = []
    if n > 0:
        sizes.append(min(4, n))
        n -= sizes[-1]
    while n > 7:
        sizes.append(8)
        n -= 8
    while n > 0:
        s = min(4, n)
        sizes.append(s)
        n -= s
    return sizes


def _build_program(plan, N, reps=1):
    """Build the (single, SPMD) Bass program for N local frames.

    reps>1 repeats the frame work (overwriting the same output) — used only
    by the dev timing harness to measure per-iteration HW time via slope.
    """
    from concourse import bacc, mybir
    import concourse.tile as tile

    f32 = mybir.dt.float32
    Alu = mybir.AluOpType

    nc = bacc.Bacc("TRN2", target_bir_lowering=False, debug=False,
                   num_devices=NCORES)
    lat_d = nc.dram_tensor("lat", [C, K * HW], f32, kind="ExternalInput")
    alpha_d = nc.dram_tensor("alpha", [C, N], f32, kind="ExternalInput")
    coef_d = nc.dram_tensor("coef", [1, K * N], f32, kind="ExternalInput")
    str_d = nc.dram_tensor("strengths", [1, K], f32, kind="ExternalInput")
    out_d = nc.dram_tensor("out", [C, N * HW], f32, kind="ExternalOutput")
    mask_d = nc.dram_tensor("mask", [1, N], f32, kind="ExternalOutput")

    pair_js = sorted({j for j in plan if isinstance(j, int)})

    with tile.TileContext(nc) as tc, ExitStack() as ctx:
        const = ctx.enter_context(tc.tile_pool(name="const", bufs=1))
        outp = ctx.enter_context(tc.tile_pool(name="outp", bufs=3))

        # alpha is read by every stt op — tiny, load first
        alpha_sb = const.tile([C, N], f32)
        nc.sync.dma_start(out=alpha_sb[:], in_=alpha_d.ap())

        # Keyframe latents: lat0+lat1 first (unblocks "Z" copies and d0),
        # then the rest in one transfer.
        lat01 = const.tile([C, 2 * HW], f32)
        nc.sync.dma_start(out=lat01[:], in_=lat_d.ap()[:, 0:2 * HW])
        lat_rest = const.tile([C, (K - 2) * HW], f32)
        nc.sync.dma_start(out=lat_rest[:], in_=lat_d.ap()[:, 2 * HW:])

        def lat_ap(k):
            if k < 2:
                return lat01[:, k * HW:(k + 1) * HW]
            return lat_rest[:, (k - 2) * HW:(k - 1) * HW]

        dma_engines = [nc.scalar, nc.sync]
        n_dma = 0

        # Masks first (tiny; DVE does them while the lat loads are in
        # flight): mask[f] = sum_k coef[k,f] * strengths[k]
        coef_sb = const.tile([1, K * N], f32)
        nc.gpsimd.dma_start(out=coef_sb[:], in_=coef_d.ap())
        str_sb = const.tile([1, K], f32)
        nc.gpsimd.dma_start(out=str_sb[:], in_=str_d.ap())
        prod = const.tile([1, K * N], f32)
        for k in range(K):
            nc.vector.tensor_scalar(
                out=prod[0:1, k * N:(k + 1) * N],
                in0=coef_sb[0:1, k * N:(k + 1) * N],
                scalar1=str_sb[0:1, k:k + 1],
                scalar2=None,
                op0=Alu.mult,
            )
        m01 = const.tile([1, N], f32)
        nc.vector.tensor_add(m01[:], prod[0:1, 0:N], prod[0:1, N:2 * N])
        m23 = const.tile([1, N], f32)
        nc.vector.tensor_add(m23[:], prod[0:1, 2 * N:3 * N], prod[0:1, 3 * N:4 * N])
        m03 = const.tile([1, N], f32)
        nc.vector.tensor_add(m03[:], m01[:], m23[:])
        mask_sb = const.tile([1, N], f32)
        nc.vector.tensor_add(mask_sb[:], m03[:], prod[0:1, 4 * N:5 * N])
        nc.gpsimd.dma_start(out=mask_d.ap(), in_=mask_sb[:])

        # stt slots, chunked for output DMA. Deltas d_j = lat[j+1] - lat[j]
        # are computed just-in-time: early pairs on DVE (in issue order,
        # right before their first stt), later pairs on Pool (idle early).
        d_t = {}
        d_pool = ctx.enter_context(tc.tile_pool(name="deltas", bufs=1))
        tmp_pool = ctx.enter_context(tc.tile_pool(name="offtmp", bufs=3))

        def get_d(j):
            if j not in d_t:
                dt_ = d_pool.tile([C, HW], f32, name=f"d{j}", tag=f"d{j}")
                eng = nc.vector if j < 2 else nc.gpsimd
                eng.tensor_tensor(out=dt_[:], in0=lat_ap(j + 1),
                                  in1=lat_ap(j), op=Alu.subtract)
                d_t[j] = dt_
            return d_t[j]

        # Measured per-[128,1024]-op rates: DVE stt ~0.75us, ACT mul ~0.75us,
        # Pool add ~2.07us. Offload a fraction of frames to the ACT->Pool
        # 2-op path so DVE and Pool finish together.
        stt_all = [f for f, key in enumerate(plan) if not isinstance(key, str)]
        n_off = int(round(len(stt_all) * 0.75 / (0.75 + 2.07)))
        # Offloaded slots skip the first (ramp) chunk, then stripe evenly:
        # ACT/Pool take every ~3rd frame so DVE and Pool finish together.
        skip = 4 if len(stt_all) >= 4 + n_off else 0
        cand = stt_all[skip:]
        off_slots = set(cand[::max(1, len(cand) // n_off)][:n_off]
                        if n_off else [])

        for _rep in range(reps):
            # Copy slots: pure DMA SBUF->DRAM from the lat tiles (no
            # compute). These fill the DMA pipe while DVE ramps.
            stt_slots = []
            for f, key in enumerate(plan):
                if key == "Z":
                    src = lat_ap(0)
                elif key == "C":
                    src = lat_ap(K - 1)
                else:
                    stt_slots.append(f)
                    continue
                # copies issue from SP only: queueing them on ACT would
                # stall ACT's tmp-muls behind the lat-load waits
                nc.sync.dma_start(
                    out=out_d.ap()[:, f * HW:(f + 1) * HW], in_=src)

            ci = 0
            for g_sz in _chunk_sizes(len(stt_slots)):
                f0 = stt_slots[ci]
                otile = outp.tile([C, g_sz * HW], f32, tag="ot")
                for g in range(g_sz):
                    f = stt_slots[ci + g]
                    assert f == f0 + g
                    j = plan[f]
                    dst = otile[:, g * HW:(g + 1) * HW]
                    if f in off_slots:
                        tmp = tmp_pool.tile([C, HW], f32, tag="tmp")
                        nc.scalar.mul(tmp[:], get_d(j)[:], alpha_sb[:, f:f + 1])
                        nc.gpsimd.tensor_tensor(out=dst, in0=tmp[:],
                                                in1=lat_ap(j), op=Alu.add)
                    else:
                        nc.vector.scalar_tensor_tensor(
                            out=dst,
                            in0=get_d(j)[:],
                            scalar=alpha_sb[:, f:f + 1],
                            in1=lat_ap(j),
                            op0=Alu.mult,
                            op1=Alu.add,
                        )
                dma_engines[n_dma % 2].dma_start(
                    out=out_d.ap()[:, f0 * HW:(f0 + g_sz) * HW], in_=otile[:])
                n_dma += 1
                ci += g_sz


    nc.compile()
    return nc


LAST_PERF = None  # BassKernelResults of the most recent run (set when _trace)


def kernel(latents, strengths, frame_indices, _trace=False):
    from concourse.bass_utils import run_bass_kernel_spmd

    global LAST_PERF
    latents = np.asarray(latents, dtype=np.float32)
    strengths = np.asarray(strengths, dtype=np.float32)
    frame_indices = np.asarray(frame_indices)

    plan, per_core_t, alpha, coef = _plan_frames(frame_indices)
    N = len(plan)

    # [C, K*HW]: partition row c holds all 5 keyframes' (contiguous) hw planes
    lat_h = np.ascontiguousarray(
        latents[:, 0].reshape(K, C, HW).transpose(1, 0, 2).reshape(C, K * HW))
    str_h = np.ascontiguousarray(strengths.reshape(1, K))

    in_maps = []
    for m in range(NCORES):
        idx = np.asarray(per_core_t[m], dtype=np.int64)
        alpha_m = np.ascontiguousarray(
            np.broadcast_to(alpha[idx][None, :], (C, N)))
        coef_m = np.ascontiguousarray(coef[:, idx].reshape(1, K * N))
        in_maps.append({
            "lat": lat_h,
            "alpha": alpha_m,
            "coef": coef_m,
            "strengths": str_h,
        })

    nc = _build_program(plan, N)
    res = run_bass_kernel_spmd(nc, in_maps, core_ids=list(range(NCORES)),
                               trace=_trace)
    if _trace:
        LAST_PERF = res

    full = np.empty((C, T, HW), np.float32)
    mask_full = np.empty(T, np.float32)
    for m in range(NCORES):
        idx = np.asarray(per_core_t[m], dtype=np.int64)
        full[:, idx, :] = res.results[m]["out"].reshape(C, N, HW)
        mask_full[idx] = res.results[m]["mask"].reshape(N)

    conditioning_latents = full.reshape(1, C, T, 32, 32)
    conditioning_masks = np.ascontiguousarray(
        np.broadcast_to(mask_full[None, :], (1, T)))
    return conditioning_latents, conditioning_masks
